# revision 11
# baseline (speedup 1.0000x reference)
"""DGL-style multi-head graph attention on 8 Trainium2 NeuronCores.

403us baseline -> 224us (TimelineSim cost model; 1.80x). What it took:
  * Degree-aware node->tile assignment (greedy bin packing with skewed
    per-tile-index edge targets): tiles 0-8 of each core hold 32 blocks
    of 128 edges, tile 9 holds 26; identical structure on every core
    (SPMD), ~1.6% edge padding vs 15% in the sorted-range baseline.
  * Inverted score matmuls: per 128-edge block PE contracts the 256
    q*k dims (lhsT=qk block, rhs=[128,8] one-hot head map), scores land
    edge-partitioned in PSUM; kills the DmaTranspose + wide-exp path.
  * Segment-sum emits hT directly (lhsT=m-slice, rhs=S): no h transpose.
  * attn stored in PAIRS [128,b,8,2] so the m-multiply's broadcast
    operand has a stride-1 innermost dim -> 2x DVE mode (the cost model
    checks only the last AP dim). m emitted in 4-block chunks so seg
    matmuls go ready incrementally and PE stays at hot p-state (bursts
    that become ready after a PE idle are all priced at 0.65GHz!).
  * q and k tables live in SBUF (row i -> partition i%128, rank i//128);
    gathers use SBUF-source dma_gather, k against its single 128-row
    rank slice. Only the v table (row-major gather, no SBUF mode) is
    written to DRAM. Removing the q write needed shallower rings
    (4,3,3,3,2,3,2) -- fits because m-chunking killed ring starvation.
  * Phase 1: v-table + out writes issued from the SP queue (ACT
    sequencer head-of-line blocking); q-side idx load demoted behind the
    first x group; PSUM rings 4-deep; copies alternate ACT/DVE.
  * out rows written bf16, low DMA priority (drain after gathers).
Cost-model floor map (per core): DMA busy 209.6us of 224 total =
gathers 172 (q 57 + k 57 + v 57, 512B/edge, bytes-exact) + v-table
write 15 + x loads 16 + idx 4 + misc; idle = 2.3 start + ~10 last-tile
tail. Engines: DVE 157, Pool 104 (gather desc-gen), PE 84, ACT 49.
Tried and NOT better: graduated last-tile mini-segs (ping-pong latency
eats the gain), staggered hT half-drains, idxd load demotion (delays
k-gather prefetch). Dead ends (all PSUM->SBUF conversion taxed):
k-dedup via one-hot matmul, project-on-gather, fused q|v gather, fp8
q/k/v (rel-err ~5-6% > 2e-2 gate). GPSIMD cannot touch PSUM (walrus
verifier rejects; CoreSim won't catch it). DMA cost model: descs x
max(elem_bytes/22.5, 7)ns /16 engines, 2x penalty below 512B/desc;
matmul priced at visit time from the pe-busy streak (>100ns mid,
>3us hot).
"""

import math
from contextlib import ExitStack

import ml_dtypes
import numpy as np

import concourse.bass as bass
import concourse.mybir as mybir
import concourse.tile as tile
from concourse import bacc, bass_utils

F32 = mybir.dt.float32
BF16 = mybir.dt.bfloat16
I16 = mybir.dt.int16

N_NODES = 10000
DIM = 256
H = 8
HD = 32
NCORES = 8
W = 128                          # node-tile width
NT = 10                          # node tiles per core
NBINS = NCORES * NT              # 80 tiles total
N_CPAD = NT * W                  # padded local nodes (1280)
N_PAD = 10240                    # padded q/v table rows (80 tiles of 128)
B_RUN = 16                       # edge blocks (of 128 edges) per inner run

MULT = mybir.AluOpType.mult
ADD = mybir.AluOpType.add
ISEQ = mybir.AluOpType.is_equal
AXX = mybir.AxisListType.X

last_results = None  # BassKernelResults of the most recent run (for test.py)


def _preprocess(src, dst):
    """Degree-aware assignment of nodes to 80 tiles of <=128 slots with
    skewed per-tile-index edge targets: tile 0..8 of each core hold ~4080
    in-edges (32 blocks), tile 9 holds ~3280 (26 blocks). Identical block
    structure across cores keeps the program SPMD."""
    import heapq

    src = np.asarray(src).astype(np.int64)
    dst = np.asarray(dst).astype(np.int64)
    deg = np.bincount(dst, minlength=N_NODES)
    order = np.argsort(-deg, kind="stable")

    def assign(targets, caps):
        bin_of = np.empty(N_NODES, np.int64)
        slot_of = np.empty(N_NODES, np.int64)
        counts = np.zeros(NBINS, np.int64)
        esum = np.zeros(NBINS, np.int64)
        # heap keyed by (esum - target): most-underfull bin first
        heap = [(-targets[b % NT], b) for b in range(NBINS)]
        heapq.heapify(heap)
        for n in order:
            d = int(deg[n])
            spill = []
            while True:
                gap, b = heapq.heappop(heap)
                if counts[b] < W and esum[b] + d <= caps[b % NT]:
                    break
                if counts[b] < W:
                    spill.append((gap, b))  # edge-cap full; may take 0-deg
            for it in spill:
                heapq.heappush(heap, it)
            bin_of[n] = b
            slot_of[n] = counts[b]
            counts[b] += 1
            esum[b] += d
            if counts[b] < W:
                heapq.heappush(heap, (esum[b] - targets[b % NT], b))
        return bin_of, slot_of, esum

    TB = [32] * (NT - 1) + [26]
    targets = [4080] * (NT - 1) + [3280]
    caps = [tb * 128 for tb in TB]
    try:
        bin_of, slot_of, esum = assign(targets, caps)
    except IndexError:
        # infeasible for this degree distribution: uniform fallback
        TB = [32] * NT
        targets = [4000] * NT
        caps = [tb * 128 for tb in TB]
        bin_of, slot_of, esum = assign(targets, caps)
    for b in range(NBINS):
        assert esum[b] <= caps[b % NT], (b, esum[b])

    node_of = np.full((NBINS, W), -1, np.int64)
    node_of[bin_of, slot_of] = np.arange(N_NODES)

    ebin = bin_of[dst]
    eslot = slot_of[dst]
    order_e = np.argsort(ebin, kind="stable")
    s_src = src[order_e]
    s_slot = eslot[order_e]
    s_bin = ebin[order_e]

    off = np.concatenate([[0], np.cumsum(TB)]) * 128  # edge offset per tile
    TOT = int(off[-1])                                # padded edges per core
    src_pad = np.zeros((NCORES, TOT), np.int64)
    kdst_pad = np.zeros((NCORES, TOT), np.int64)
    dstloc_pad = np.full((NCORES, TOT), -1.0, np.float32)

    bounds = np.searchsorted(s_bin, np.arange(NBINS + 1))
    for bb in range(NBINS):
        c, t = divmod(bb, NT)
        lo, hi = bounds[bb], bounds[bb + 1]
        n = hi - lo
        assert n <= off[t + 1] - off[t]
        o = int(off[t])
        src_pad[c, o:o + n] = s_src[lo:hi]
        kdst_pad[c, o:o + n] = s_slot[lo:hi]  # rank-local row (tile t's rank)
        dstloc_pad[c, o:o + n] = s_slot[lo:hi].astype(np.float32)

    def tile_idx(a):
        # sequence -> dma_gather layout [128, S/16]: row p holds seq[s*16 + p%16]
        seq = a.reshape(-1, 16).T.astype(np.int16)       # [16, S/16]
        return np.ascontiguousarray(np.tile(seq, (8, 1)))  # [128, S/16]

    idx_src = np.stack([tile_idx(src_pad[c]) for c in range(NCORES)])
    idx_dst = np.stack([tile_idx(kdst_pad[c]) for c in range(NCORES)])
    # dstloc in block-major gather layout: [e, blk] = dstloc[blk*128 + e]
    nblk = TOT // 128
    dstloc = np.stack([
        np.ascontiguousarray(
            dstloc_pad[c].reshape(nblk, 128).T)
        for c in range(NCORES)])
    return tuple(TB), idx_src, idx_dst, dstloc, node_of


_prog_cache = {}


def _build(TB):
    import os
    skip = set(os.environ.get("KERNEL_SKIP", "").split(","))
    RB = [int(x) for x in os.environ.get(
        "KERNEL_RINGS", "4,3,3,3,2,3,2").split(",")]  # gatk,gatq,gatv,S,qk,m,stage
    VAR = set(os.environ.get("KERNEL_VAR", "").split(","))
    NBLK = sum(TB)
    OFFB = [sum(TB[:t]) for t in range(NT)]
    GRAD = os.environ.get("KERNEL_GRAD", "")
    def tile_segs(t):
        segs, b0 = [], 0
        while b0 < TB[t]:
            nb = min(B_RUN, TB[t] - b0)
            segs.append((b0, nb))
            b0 += nb
        if t == NT - 1 and GRAD:
            tail = [int(x) for x in GRAD.split(",")]
            if sum(tail) < TB[t]:
                segs = [(0, TB[t] - sum(tail))]
                b0 = TB[t] - sum(tail)
                for nb in tail:
                    segs.append((b0, nb))
                    b0 += nb
        return segs
    SEG_SIZES = sorted({nb for t in range(NT) for _, nb in tile_segs(t)})
    SEQ = NBLK * 128
    nc = bacc.Bacc("TRN2", target_bir_lowering=False, debug=False)

    xT_d = nc.dram_tensor("xT", [DIM, N_PAD], BF16, kind="ExternalInput").ap()
    xlocT_d = nc.dram_tensor("xlocT", [DIM, N_CPAD], BF16, kind="ExternalInput").ap()
    wqvT_d = nc.dram_tensor("wqvT", [DIM, 2 * DIM], BF16, kind="ExternalInput").ap()
    wkT_d = nc.dram_tensor("wkT", [DIM, DIM], BF16, kind="ExternalInput").ap()
    woT_d = nc.dram_tensor("woT", [DIM, DIM], BF16, kind="ExternalInput").ap()
    idxs_d = nc.dram_tensor("idx_src", [128, SEQ // 16], I16, kind="ExternalInput").ap()
    idxd_d = nc.dram_tensor("idx_dst", [128, SEQ // 16], I16, kind="ExternalInput").ap()
    dstloc_d = nc.dram_tensor("dstloc", [128, NBLK], BF16, kind="ExternalInput").ap()
    bd8lo_d = nc.dram_tensor("bd8lo", [128, 8], BF16, kind="ExternalInput").ap()
    bd8hi_d = nc.dram_tensor("bd8hi", [128, 8], BF16, kind="ExternalInput").ap()
    out_d = nc.dram_tensor("out", [N_CPAD, DIM], BF16, kind="ExternalOutput").ap()

    with ExitStack() as ctx:
        tc = ctx.enter_context(tile.TileContext(nc))
        consts = ctx.enter_context(tc.tile_pool(name="consts", bufs=1))

        def load_w(name, d_ap):
            sb = consts.tile([128, 2, d_ap.shape[1]], d_ap.dtype, name=name)
            nc.sync.dma_start(sb[:], d_ap.rearrange("(a p) i -> p a i", p=128))
            return sb

        wk_sb = load_w("wk_sb", wkT_d)
        idxd_sb = consts.tile([128, SEQ // 16], I16)
        nc.sync.dma_start(idxd_sb[:], idxd_d)
        wqv_sb = load_w("wqv_sb", wqvT_d)
        idxs_sb = consts.tile([128, SEQ // 16], I16)
        wo_sb = load_w("wo_sb", woT_d)
        bd8lo = consts.tile([128, 8], BF16)
        bd8hi = consts.tile([128, 8], BF16)
        iotab_sb = consts.tile([128, 128 * B_RUN], BF16)
        dstloc_sb = consts.tile([128, NBLK], BF16)

        # SBUF-resident k and q tables (row i -> partition i%128, rank i//128)
        k_table = consts.tile([128, NT, DIM], BF16)
        q_table = consts.tile([128, N_PAD // 128, DIM], BF16)

        dram = ctx.enter_context(tc.tile_pool(name="dram", bufs=1, space="DRAM"))
        v_table = dram.tile([N_PAD, DIM], BF16)

        nidx_regs = {}
        for nb in SEG_SIZES:
            nidx_regs[nb] = nc.alloc_register(mybir.EngineType.Pool,
                                              f"nidx_reg{nb}")
            nc.gpsimd.reg_mov(nidx_regs[nb], nb * 128)
        gatk = ctx.enter_context(tc.tile_pool(name="gatk", bufs=RB[0]))

        # ---- phase 1: projection tables ----
        with tc.tile_pool(name="pin", bufs=3) as pin, \
             tc.tile_pool(name="pps", bufs=4, space="PSUM") as pps, \
             tc.tile_pool(name="pout", bufs=3) as pout, \
             tc.tile_pool(name="pov", bufs=5) as pov:

            if "phase1" not in skip:
                # k: local projection straight into the SBUF table
                xk = xlocT_d.rearrange("(a p) (g t w) -> p a g t w",
                                       p=128, w=128, t=5)
                for g in range(2):
                    xt = pin.tile([128, 2, 5, 128], BF16, tag="xt")
                    nc.sync.dma_start(xt[:], xk[:, :, g, :, :])
                    for t in range(5):
                        ps = pps.tile([128, DIM], F32, tag="psq")
                        nc.tensor.matmul(ps[:], xt[:, 0, t, :], wk_sb[:, 0, :],
                                         start=True, stop=False)
                        nc.tensor.matmul(ps[:], xt[:, 1, t, :], wk_sb[:, 1, :],
                                         start=False, stop=True)
                        if t % 2 == 0:
                            nc.scalar.copy(k_table[:, g * 5 + t, :], ps[:])
                        else:
                            nc.vector.tensor_copy(k_table[:, g * 5 + t, :], ps[:])

                # separate q and v DRAM tables; q written first per group so
                # q gathers can start before the v table completes
                GRP = 16
                x4 = xT_d.rearrange("(a p) (g t w) -> p a g t w",
                                    p=128, w=128, t=GRP)
                tbv = v_table[:].rearrange("(g t p) w -> p g t w", p=128, t=GRP)
                for g in range(N_PAD // 128 // GRP):
                    xt = pin.tile([128, 2, GRP, 128], BF16, tag="xt")
                    nc.sync.dma_start(xt[:], x4[:, :, g, :, :])
                    if g == 0:
                        # the q-side idx table loads behind the first x group:
                        # not needed until q gathers start (well after phase 1)
                        nc.sync.dma_start(idxs_sb[:], idxs_d)
                    obv = pov.tile([128, GRP, DIM], BF16, tag="obv")
                    for t in range(GRP):
                        psq = pps.tile([128, DIM], F32, tag="psq")
                        nc.tensor.matmul(psq[:], xt[:, 0, t, :],
                                         wqv_sb[:, 0, 0:DIM],
                                         start=True, stop=False)
                        nc.tensor.matmul(psq[:], xt[:, 1, t, :],
                                         wqv_sb[:, 1, 0:DIM],
                                         start=False, stop=True)
                        psv = pps.tile([128, DIM], F32, tag="psv")
                        nc.tensor.matmul(psv[:], xt[:, 0, t, :],
                                         wqv_sb[:, 0, DIM:2 * DIM],
                                         start=True, stop=False)
                        nc.tensor.matmul(psv[:], xt[:, 1, t, :],
                                         wqv_sb[:, 1, DIM:2 * DIM],
                                         start=False, stop=True)
                        if t % 2 == 0:
                            nc.scalar.copy(q_table[:, g * GRP + t, :], psq[:])
                            nc.vector.tensor_copy(obv[:, t, :], psv[:])
                        else:
                            nc.vector.tensor_copy(q_table[:, g * GRP + t, :],
                                                  psq[:])
                            nc.scalar.copy(obv[:, t, :], psv[:])
                    nc.sync.dma_start(tbv[:, g, :, :], obv[:])

            nc.sync.dma_start(bd8lo[:], bd8lo_d)
            nc.sync.dma_start(bd8hi[:], bd8hi_d)
            nc.gpsimd.iota(iotab_sb[:].rearrange("p (n b) -> p n b", b=B_RUN),
                           [[1, 128], [0, B_RUN]], channel_multiplier=0,
                           allow_small_or_imprecise_dtypes=True)
            nc.sync.dma_start(dstloc_sb[:], dstloc_d)

        # ---- phase 2: per node-tile edge processing ----
        with tc.tile_pool(name="gatq", bufs=RB[1]) as gatq, \
             tc.tile_pool(name="gatv", bufs=RB[2]) as gatv, \
             tc.tile_pool(name="spool", bufs=RB[3]) as spool, \
             tc.tile_pool(name="qkpool", bufs=RB[4]) as qkpool, \
             tc.tile_pool(name="mpool", bufs=RB[5]) as mpool, \
             tc.tile_pool(name="small", bufs=4) as small, \
             tc.tile_pool(name="hps", bufs=2, space="PSUM") as hps, \
             tc.tile_pool(name="scps", bufs=2, space="PSUM") as scps, \
             tc.tile_pool(name="tps", bufs=2, space="PSUM") as tps, \
             tc.tile_pool(name="stage", bufs=RB[6]) as stage:

            for t in range(NT):
                segs = tile_segs(t)
                # [128, 2, 512]: each a-half owns a full PSUM bank so the
                # two interleaved accumulation groups don't collide
                hT_ps = hps.tile([128, 2, 512], F32, tag="h")
                h_ps = [hT_ps[:, 0, 0:128], hT_ps[:, 1, 0:128]]
                kgs = []
                for blk0, nb in segs:
                    col0 = (OFFB[t] + blk0) * 8
                    kT_g = gatk.tile([128, 2, nb * 128], BF16, tag="kTg")
                    if "gather" not in skip:
                        nc.gpsimd.dma_gather(kT_g[:], k_table[:, t, :],
                                             idxd_sb[:, col0:col0 + nb * 8],
                                             nb * 128, nidx_regs[nb], DIM,
                                             transpose=True, single_packet=False,
                                             sbuf_tokens_per_rank=128,
                                             sbuf_free_dim_per_rank=2 * DIM)
                    kgs.append(kT_g)
                # stage 1: S builds (consts only) + q/v gather issuance
                Ss, qgs, vgs = [], [], []
                for blk0, nb in segs:
                    col0 = (OFFB[t] + blk0) * 8
                    S = spool.tile([128, 128, nb], BF16, tag="S")
                    nc.vector.tensor_tensor(
                        S[:],
                        iotab_sb[:].rearrange("p (n b) -> p n b", b=B_RUN)
                            [:, :, 0:nb],
                        dstloc_sb[:, OFFB[t] + blk0:OFFB[t] + blk0 + nb]
                            .unsqueeze(1).broadcast_to((128, 128, nb)),
                        op=ISEQ)
                    Ss.append(S)
                    qT_g = gatq.tile([128, 2, nb * 128], BF16, tag="qTg")
                    v_g = gatv.tile([128, nb, DIM], BF16, tag="vg")
                    if "gather" not in skip:
                        nc.gpsimd.dma_gather(qT_g[:], q_table[:],
                                             idxs_sb[:, col0:col0 + nb * 8],
                                             nb * 128, nidx_regs[nb], DIM,
                                             transpose=True,
                                             single_packet=False,
                                             sbuf_tokens_per_rank=128,
                                             sbuf_free_dim_per_rank=2 * DIM)
                        nc.gpsimd.dma_gather(v_g[:], v_table[:],
                                             idxs_sb[:, col0:col0 + nb * 8],
                                             nb * 128, nidx_regs[nb], DIM,
                                             single_packet=False)
                    qgs.append(qT_g)
                    vgs.append(v_g)

                if "compute" in skip:
                    o_sb = stage.tile([128, DIM], BF16, tag="o_sb")
                    nc.vector.memset(o_sb[:], 0.0)
                    nc.scalar.dma_start(out_d[t * 128:(t + 1) * 128, :], o_sb[:])
                    continue

                # stage 2: qk products + scores + exp for all runs, so the
                # in-order DVE frees gather rings before the softmax chain
                escs = []
                for r, (blk0, nb) in enumerate(segs):
                    qkT = qkpool.tile([128, 2, nb * 128], BF16, tag="qkT")
                    nc.vector.tensor_tensor(qkT[:], qgs[r][:], kgs[r][:], op=MULT)
                    sc_ps = scps.tile([128, nb, H], F32, tag="sc")
                    for b in range(nb):
                        nc.tensor.matmul(sc_ps[:, b, :],
                                         qkT[:, 0, b * 128:(b + 1) * 128],
                                         bd8lo[:], start=True, stop=False)
                        nc.tensor.matmul(sc_ps[:, b, :],
                                         qkT[:, 1, b * 128:(b + 1) * 128],
                                         bd8hi[:], start=False, stop=True)
                    esc_e = small.tile([128, nb, H], BF16, tag="esce")
                    nc.scalar.activation(esc_e[:], sc_ps[:],
                                         func=mybir.ActivationFunctionType.Exp,
                                         scale=1.0 / math.sqrt(HD))
                    escs.append(esc_e)

                # stage 3: softmax normalize + messages
                ms = []
                for r, (blk0, nb) in enumerate(segs):
                    esc_e = escs[r]
                    z = small.tile([128, nb], F32, tag="z")
                    nc.vector.tensor_reduce(z[:], esc_e[:], axis=AXX, op=ADD)
                    zr = small.tile([128, nb], F32, tag="zr")
                    nc.vector.reciprocal(zr[:], z[:])
                    # attn materialized in PAIRS so the m-multiply's
                    # broadcast operand has a stride-1 innermost dim (2x DVE)
                    attn = small.tile([128, nb, H, 2], BF16, tag="at")
                    nc.vector.tensor_tensor(
                        attn[:],
                        esc_e[:].unsqueeze(3).broadcast_to((128, nb, H, 2)),
                        zr[:].unsqueeze(2).unsqueeze(3)
                            .broadcast_to((128, nb, H, 2)), op=MULT)
                    m = mpool.tile([128, nb, DIM], BF16, tag="m")
                    for c0 in range(0, nb, 4):  # chunked so seg matmuls become
                        MC = min(4, nb - c0)    # ready early (PE streak stays hot)
                        nc.vector.tensor_tensor(
                            m[:, c0:c0 + MC, :]
                                .rearrange("p b (h x two) -> p b h x two",
                                           h=H, two=2),
                            vgs[r][:, c0:c0 + MC, :]
                                .rearrange("p b (h x two) -> p b h x two",
                                           h=H, two=2),
                            attn[:, c0:c0 + MC, :, :].unsqueeze(3)
                                .broadcast_to((128, MC, H, HD // 2, 2)),
                            op=MULT)
                    ms.append(m)
                if "compute" not in skip:
                    # swapped operands: hT[d, j] = sum_e m[e, d] S[e, j] --
                    # the h tile comes out pre-transposed for the Wo matmul
                    nseg = len(segs)
                    for r, (blk0, nb) in enumerate(segs):
                        for b in range(nb):
                            for a in range(2):
                                nc.tensor.matmul(
                                    h_ps[a],
                                    ms[r][:, b, a * 128:(a + 1) * 128],
                                    Ss[r][:, :, b],
                                    start=(r == 0 and b == 0),
                                    stop=(r == nseg - 1 and b == nb - 1))

                hT_sb = stage.tile([128, 2, 128], BF16, tag="hT_sb")
                nc.scalar.copy(hT_sb[:], hT_ps[:, :, 0:128])
                o_ps = tps.tile([128, DIM], F32, tag="o")
                for a in range(2):
                    nc.tensor.matmul(o_ps[:], hT_sb[:, a, :], wo_sb[:, a, :],
                                     start=(a == 0), stop=(a == 1))
                o_sb = stage.tile([128, DIM], BF16, tag="o_sb")
                nc.scalar.copy(o_sb[:], o_ps[:])
                wout = nc.sync.dma_start(out_d[t * 128:(t + 1) * 128, :], o_sb[:])
                wout.bass_priority = 1 << 24  # drain after all gathers

    nc.compile()
    return nc


def _bd8_mat(base):
    bd = np.zeros((128, 8), np.float32)
    for d in range(128):
        bd[d, base + d // HD] = 1.0
    return bd.astype(ml_dtypes.bfloat16)


def _make_in_maps(x, Wq, Wk, Wv, Wo, idx_src, idx_dst, dstloc, node_of):
    x = np.asarray(x, np.float32)
    xp = np.zeros((N_PAD, DIM), np.float32)
    xp[:N_NODES] = x
    xT = np.ascontiguousarray(xp.T.astype(ml_dtypes.bfloat16))
    wqvT = np.ascontiguousarray(np.concatenate(
        [np.asarray(Wq, np.float32).T, np.asarray(Wv, np.float32).T],
        axis=1).astype(ml_dtypes.bfloat16))
    wkT = np.ascontiguousarray(np.asarray(Wk, np.float32).T
                               .astype(ml_dtypes.bfloat16))
    woT = np.ascontiguousarray(np.asarray(Wo, np.float32).T
                               .astype(ml_dtypes.bfloat16))
    in_maps = []
    for c in range(NCORES):
        # xloc row (t*128 + s) = x[node_of[c*NT + t, s]] (zeros for pads)
        nodes = node_of[c * NT:(c + 1) * NT].reshape(-1)
        xl = np.zeros((N_CPAD, DIM), np.float32)
        valid = nodes >= 0
        xl[valid] = x[nodes[valid]]
        in_maps.append({
            "xT": xT,
            "xlocT": np.ascontiguousarray(xl.T.astype(ml_dtypes.bfloat16)),
            "wqvT": wqvT, "wkT": wkT, "woT": woT,
            "idx_src": idx_src[c], "idx_dst": idx_dst[c],
            "dstloc": dstloc[c].astype(ml_dtypes.bfloat16),
            "bd8lo": _bd8_mat(0), "bd8hi": _bd8_mat(4),
        })
    return in_maps


def kernel(x, src, dst, Wq, bq, Wk, bk, Wv, bv, Wo, bo, **_unused):
    global last_results
    assert abs(np.asarray(bq)).max() == 0 and abs(np.asarray(bk)).max() == 0 \
        and abs(np.asarray(bv)).max() == 0, "nonzero qkv biases unsupported"

    TB, idx_src, idx_dst, dstloc, node_of = _preprocess(src, dst)
    if TB not in _prog_cache:
        _prog_cache[TB] = _build(TB)
    nc = _prog_cache[TB]
    in_maps = _make_in_maps(x, Wq, Wk, Wv, Wo, idx_src, idx_dst, dstloc, node_of)

    import os
    trace = bool(int(os.environ.get("KERNEL_TRACE", "0")))
    res = bass_utils.run_bass_kernel_spmd(
        nc, in_maps, core_ids=list(range(NCORES)), trace=trace)
    last_results = res

    out = np.empty((N_NODES, DIM), np.float32)
    for c in range(NCORES):
        nodes = node_of[c * NT:(c + 1) * NT].reshape(-1)
        valid = nodes >= 0
        out[nodes[valid]] = res.results[c]["out"][valid]
    out += np.asarray(bo, np.float32)[None, :]
    return out
